# revision 69
# baseline (speedup 1.0000x reference)
"""Fused QKV + RMSNorm + RoPE + self-attention kernel for Trainium2.

Sharding: tensor-parallel over heads. 16 heads / 8 cores = 2 heads per core.
Each core computes qkv projection for its 2 heads (column-parallel on the
3*dim output), per-head RMSNorm/RoPE/attention locally, and writes its
unnormalized AV accumulator [B, HPC, D, N] plus softmax denominators.
The host divides, transposes to token-major and concatenates head slices
(the output projection is absent, so the "all-gather" is a host-side concat).

Host-side weight preprocessing:
  - the reference layout interleaves q/k/v at stride 3 per (head, dim):
    row = h*384 + d*3 + j.  We de-interleave by permuting w_qkv rows.
  - q/k head-dims are permuted even-first ([0,2,..,126,1,3,..,127]) so the
    interleaved RoPE becomes rotate-half style.  Scores q.k are invariant
    under a common permutation of q and k head-dims (RMSNorm too), and v is
    left unpermuted, so the final output is unchanged.

Compute dtype: bf16 matmuls with fp32 accumulation; softmax/statistics fp32.
"""

import sys

sys.path.insert(0, "/opt/trn_rl_repo")

import numpy as np
import ml_dtypes

import concourse.bass as bass
import concourse.mybir as mybir
import concourse.tile as tile
from concourse import bacc
from concourse.masks import make_identity

B = 2
SEQ = 2048
DIM = 2048
NHEADS = 16
HEAD_DIM = 128
NCORES = 8
HPC = NHEADS // NCORES  # heads per core = 2
EPS = 1e-6
SCALE = float(HEAD_DIM) ** -0.5
P = 128  # partitions

F32 = mybir.dt.float32
BF16 = mybir.dt.bfloat16
I32 = mybir.dt.int32

QG = 512  # q tokens per attention inner group


def build_nc(seq=SEQ, batches=B):
    """Build the SPMD per-core graph. Same graph runs on all 8 cores."""
    tokens = batches * seq
    ntb = seq // P  # token tiles per batch (16)
    kc_n = DIM // P  # contraction chunks for qkv projection (16)
    fpc = 3 * HPC * HEAD_DIM  # per-core projection output features = 768
    qg_per = seq // QG  # q groups per (b, h) (4)
    gq = QG // P  # 128-tiles per q group (4)

    nc = bacc.Bacc(None, target_bir_lowering=False)

    xt_ext = nc.declare_dram_parameter("xt", [DIM, tokens], BF16, isOutput=False)
    wt_ext = nc.declare_dram_parameter("wt", [DIM, fpc], BF16, isOutput=False)
    bias_ext = nc.declare_dram_parameter("bias", [1, fpc], F32, isOutput=False)
    cs_ext = nc.declare_dram_parameter("cs", [seq, 128], BF16, isOutput=False)
    sc_ext = nc.declare_dram_parameter("sc", [seq, 128], BF16, isOutput=False)
    av_ext = nc.declare_dram_parameter(
        "av", [batches, HPC, HEAD_DIM, seq], F32, isOutput=True
    )
    sums_ext = nc.declare_dram_parameter(
        "sums", [batches, HPC, P, ntb], F32, isOutput=True
    )

    add = mybir.AluOpType.add
    sub = mybir.AluOpType.subtract
    mul = mybir.AluOpType.mult

    with tile.TileContext(nc) as tc:
        with (
            tc.tile_pool(name="consts", bufs=1) as consts,
            tc.tile_pool(name="persist", bufs=1) as persist,
        ):
            ones_col = consts.tile([P, 1], BF16, tag="ones")
            nc.vector.memset(ones_col[:], 1.0)
            ident = consts.tile([P, P], BF16, tag="ident")
            make_identity(nc, ident[:])

            # consts stream on the GpSimd (SWDGE) DMA queue so the x-tile
            # loads on the Sync queue are never stuck behind them
            # consts tiles allocated here; their DMAs are emitted inside
            # phase1_tile(0,0) after the first x chunks so the very first
            # matmul's inputs hit the DMA engines first
            wt_sb = consts.tile([P, kc_n, fpc], BF16, tag="wt")
            wt_r = wt_ext[:].rearrange("(kc p) f -> p kc f", p=P)
            bias_sb = consts.tile([P, fpc], F32, tag="bias")
            cs_sb = consts.tile([P, ntb, P], BF16, tag="cs")
            sc_sb = consts.tile([P, ntb, P], BF16, tag="sc")

            def emit_const_dmas():
                # wt0-3 on the HWDGE sync queue (the gpsimd SWDGE path pays
                # a ~6us Q7 IRAM warmup on its first descriptor); more than 4
                # here delays the x-tile stream behind weight bytes
                for kc in range(4):
                    nc.sync.dma_start(out=wt_sb[:, kc, :], in_=wt_r[:, kc, :])
                for kc in range(4, kc_n):
                    nc.gpsimd.dma_start(out=wt_sb[:, kc, :], in_=wt_r[:, kc, :])
                bap = bias_ext[:]
                bias_bcast = bass.AP(
                    tensor=bap.tensor, offset=bap.offset, ap=[[0, P], [1, fpc]]
                )
                nc.gpsimd.dma_start(out=bias_sb[:], in_=bias_bcast)
                cs_r = cs_ext[:].rearrange("(ti p) d -> p ti d", p=P)
                sc_r = sc_ext[:].rearrange("(ti p) d -> p ti d", p=P)
                nc.gpsimd.dma_start(out=cs_sb[:], in_=cs_r)
                nc.gpsimd.dma_start(out=sc_sb[:], in_=sc_r)

            # persistent per-(batch, local-head) attention operands
            # qT/kT feature-major: [d, tile, tok]; v token-major: [tok, chunk, d]
            qT = {}
            kT = {}
            vv = {}
            for b in range(batches):
                for hl in range(HPC):
                    qT[(b, hl)] = persist.tile([P, ntb, P], BF16, tag=f"qT{b}_{hl}", name=f"qT{b}_{hl}")
                    kT[(b, hl)] = persist.tile([P, ntb, P], BF16, tag=f"kT{b}_{hl}", name=f"kT{b}_{hl}")
                    vv[(b, hl)] = persist.tile([P, ntb, P], BF16, tag=f"v{b}_{hl}", name=f"v{b}_{hl}")
            sums_sb = {}
            for b in range(batches):
                for hl in range(HPC):
                    sums_sb[(b, hl)] = persist.tile(
                        [P, ntb], F32, tag=f"sums{b}_{hl}", name=f"sums{b}_{hl}"
                    )

            # Phase 1 (projection+norm+rope) and phase 2 (attention) share
            # pools and are emitted interleaved per batch, so batch b+1's
            # PE-heavy projection overlaps batch b's ACT-heavy softmax.
            with (
                tc.tile_pool(name="p1", bufs=2) as p1,
                tc.tile_pool(name="p1s", bufs=2) as p1s,
                tc.tile_pool(name="p2", bufs=2) as p2,
                tc.tile_pool(name="psp2", bufs=1, space="PSUM") as psp2,
            ):
                xt_r = xt_ext[:].rearrange("(kc p) n -> p kc n", p=P)
                # psp1 (psA/psB/tp, 4 banks) lives only while phase 1 runs;
                # its banks are recycled into psp3 (a 4-bank [P,2048] scores
                # tile) for the batch-1 tail, enabling 2048-wide exp calls
                pools = {}

                def phase1_tile(b_idx, ti):
                    t = b_idx * ntb + ti
                    # bufs=3: with 2, tile t+2's x-load can only ISSUE once
                    # tile t's matmuls release the slot, putting the DMA
                    # latency on the critical path during the cold start
                    x_tile = p1.tile(
                        [P, kc_n, P], BF16, tag="x", bufs=3, name="x_tile"
                    )
                    qc = kc_n // 4
                    for xq in range(4):
                        nc.sync.dma_start(
                            out=x_tile[:, xq * qc : (xq + 1) * qc, :],
                            in_=xt_r[:, xq * qc : (xq + 1) * qc, t * P : (t + 1) * P],
                        )
                        if b_idx == 0 and ti == 0 and xq == 0:
                            emit_const_dmas()
                    ps_a = pools["psp1"].tile([P, 512], F32, tag="psA", bufs=2, name="ps_a")
                    ps_b = pools["psp1"].tile([P, 256], F32, tag="psB", bufs=1, name="ps_b")
                    for kc in range(kc_n):
                        st = kc == 0
                        sp = kc == kc_n - 1
                        nc.tensor.matmul(
                            ps_a[:],
                            x_tile[:, kc, :],
                            wt_sb[:, kc, 0:512],
                            start=st,
                            stop=sp,
                        )
                        nc.tensor.matmul(
                            ps_b[:],
                            x_tile[:, kc, :],
                            wt_sb[:, kc, 512:768],
                            start=st,
                            stop=sp,
                        )
                    # evac + bias add; qkv in bf16 so downstream DVE runs 2x
                    qkv_sb = p1.tile([P, fpc], BF16, tag="qkv")
                    nc.vector.tensor_tensor(
                        qkv_sb[:, 0:512], ps_a[:], bias_sb[:, 0:512], add
                    )
                    nc.vector.tensor_tensor(
                        qkv_sb[:, 512:768], ps_b[:], bias_sb[:, 512:768], add
                    )

                    # rms stats for the 4 q/k blocks (sq is a dummy out).
                    # ACT is idle while batch 0 projects (no softmax yet), so
                    # b0 stats run there; b1 stats stay on DVE.
                    ms = p1s.tile([P, 4], F32, tag="ms")
                    sq = p1s.tile([P, P], BF16, tag="sq", bufs=1)
                    for blk in range(4):
                        xb = qkv_sb[:, blk * P : (blk + 1) * P]
                        if b_idx == 0:
                            nc.scalar.activation(
                                out=sq[:],
                                in_=xb,
                                func=mybir.ActivationFunctionType.Square,
                                accum_out=ms[:, blk : blk + 1],
                            )
                        else:
                            nc.vector.scalar_tensor_tensor(
                                sq[:], xb, 1.0, xb, mul, mul,
                                accum_out=ms[:, blk : blk + 1],
                            )
                    # rstd = 1/sqrt(ms/128 + eps) via bit-trick + one Newton
                    # step on DVE (keeps ACT exp-only)
                    aa = p1s.tile([P, 4], F32, tag="aa")
                    nc.vector.tensor_scalar(
                        aa[:], ms[:], 1.0 / HEAD_DIM, EPS, mul, add
                    )
                    y0i = p1s.tile([P, 4], I32, tag="y0i")
                    nc.vector.tensor_scalar(
                        y0i[:], aa[:].bitcast(I32), 1, None,
                        mybir.AluOpType.logical_shift_right,
                    )
                    nc.vector.tensor_scalar(
                        y0i[:], y0i[:], -1, 0x5F3759DF, mul, add
                    )
                    y0 = y0i[:].bitcast(F32)
                    t1 = p1s.tile([P, 4], F32, tag="t1")
                    nc.vector.tensor_tensor(t1[:], y0, y0, mul)
                    nc.vector.scalar_tensor_tensor(
                        t1[:], t1[:], -0.5, aa[:], mul, mul
                    )
                    rstd = p1s.tile([P, 4], F32, tag="rstd")
                    nc.vector.scalar_tensor_tensor(
                        rstd[:], t1[:], 1.5, y0, add, mul
                    )
                    # second Newton step for accuracy
                    nc.vector.tensor_tensor(t1[:], rstd[:], rstd[:], mul)
                    nc.vector.scalar_tensor_tensor(
                        t1[:], t1[:], -0.5, aa[:], mul, mul
                    )
                    nc.vector.scalar_tensor_tensor(
                        rstd[:], t1[:], 1.5, rstd[:], add, mul
                    )

                    # wide rope over all 4 q/k blocks at once:
                    #   m1 = qk * [c|s]x4   m2 = qk * [-s|c]x4
                    #   roped = per-block [m1_lo - m2_lo_pair ...] via one
                    #   strided subtract (4D APs)
                    csb = cs_sb[:, ti, :]
                    scb = sc_sb[:, ti, :]
                    cs_rep = bass.AP(
                        tensor=csb.tensor, offset=csb.offset,
                        ap=[list(csb.ap[0]), [0, 4], [1, P]],
                    )
                    sc_rep = bass.AP(
                        tensor=scb.tensor, offset=scb.offset,
                        ap=[list(scb.ap[0]), [0, 4], [1, P]],
                    )
                    m12 = p1.tile([P, 2, 4, P], BF16, tag="m12")
                    qk_in = qkv_sb[:, 0:512].rearrange("p (r c) -> p r c", r=4)
                    nc.vector.tensor_tensor(m12[:, 0], qk_in, cs_rep, mul)
                    nc.vector.tensor_tensor(m12[:, 1], qk_in, sc_rep, mul)
                    mb = m12[:]
                    # a: [x1c | x2c] per block; b: [x2s | -x1s] per block
                    a_ap = bass.AP(
                        tensor=mb.tensor, offset=mb.offset,
                        ap=[list(mb.ap[0]), [P, 4], [576, 2], [1, 64]],
                    )
                    b_ap = bass.AP(
                        tensor=mb.tensor, offset=mb.offset + 64,
                        ap=[list(mb.ap[0]), [P, 4], [448, 2], [1, 64]],
                    )
                    roped = p1.tile([P, 4, 2, 64], BF16, tag="roped")
                    nc.vector.tensor_tensor(roped[:], a_ap, b_ap, sub)

                    # normalize (q and k) + transpose via DMA xbar into
                    # feature-major persistent tiles
                    rview = roped[:].rearrange("p b a c -> p (b a c)")
                    norm_sb = p1.tile([P, 512], BF16, tag="norm")
                    for blk in range(4):
                        c0 = blk * P
                        nc.vector.tensor_scalar_mul(
                            norm_sb[:, c0 : c0 + P],
                            rview[:, c0 : c0 + P],
                            rstd[:, blk : blk + 1],
                        )
                    # transposes + v copies are deferred: emitted after the
                    # NEXT tile's matmuls so the PE never queues a transpose
                    # that waits on this tile's still-running DVE chain
                    def part_b():
                        for blk in range(4):
                            dest = qT if blk < 2 else kT
                            hl = blk % 2
                            tp = pools["psp1"].tile(
                                [P, P], BF16, tag="tp", bufs=1, name="tp"
                            )
                            nc.tensor.transpose(
                                tp[:],
                                norm_sb[:, blk * P : (blk + 1) * P],
                                ident[:],
                            )
                            nc.vector.tensor_copy(
                                dest[(b_idx, hl)][:, ti, :], tp[:]
                            )
                        for hl in range(HPC):
                            c0 = 512 + hl * P
                            nc.gpsimd.tensor_copy(
                                vv[(b_idx, hl)][:, ti, :], qkv_sb[:, c0 : c0 + P]
                            )

                    return part_b

                def emit_exp_pair(probsT, k_t, qs_ap, kc, alt=False):
                    if alt:
                        s_ps = pools["psp3"].tile(
                            [P, 1024], F32, tag="spsB2", bufs=1, name="s_ps2"
                        )
                    else:
                        s_ps = psp2.tile(
                            [P, 1024], F32, tag="spsB", bufs=1, name="s_ps"
                        )
                    nc.tensor.matmul(
                        s_ps[:, 0:512], k_t[:, kc, :], qs_ap,
                        start=True, stop=True,
                    )
                    nc.tensor.matmul(
                        s_ps[:, 512:1024], k_t[:, kc + 1, :], qs_ap,
                        start=True, stop=True,
                    )
                    nc.scalar.activation(
                        out=probsT[:, kc : kc + 2, :],
                        in_=s_ps[:],
                        func=mybir.ActivationFunctionType.Exp,
                        scale=SCALE,
                    )

                def phase2_qgroup(b, hl, qg, last=False, big=False, tail_scol=False):
                    q_t = qT[(b, hl)]
                    k_t = kT[(b, hl)]
                    v_t = vv[(b, hl)]
                    qs_ap = q_t[:, qg * gq : (qg + 1) * gq, :]
                    probsT = p2.tile([P, ntb, QG], BF16, tag="probsT", bufs=3, name="probsT")

                    def emit_half_fold(pT, half):
                        # fold chunks [8h, 8h+8) down to [P, 512]
                        cur = pT[:, 8 * half : 8 * half + 8, :].rearrange(
                            "p a b -> p (a b)"
                        )
                        width = 8 * QG
                        lvl = 1
                        while width > QG:
                            width //= 2
                            nxt = p2.tile(
                                [P, width], BF16, tag=f"fold{lvl}",
                                bufs=(2 if width == QG else 1), name="hfold",
                            )
                            nc.vector.tensor_tensor(
                                nxt[:], cur[:, 0:width],
                                cur[:, width : 2 * width], add,
                            )
                            cur = nxt[:]
                            lvl += 1
                        return cur
                    # Middle: big(1024)/small(512) exp ping-pong — the small
                    # slot keeps ACT busy while PE refills the single big
                    # slot. Tail: two alternating 1024 slots (recycled
                    # phase-1 banks) make every exp a big one (8 pairs), the
                    # cheapest per-element ACT pattern.
                    half1 = None
                    if tail_scol:
                        for pr in range(ntb // 2):
                            emit_exp_pair(
                                probsT, k_t, qs_ap, 2 * pr, alt=(pr % 2 == 1)
                            )
                            if last and pr == ntb // 4 - 1:
                                # first half-tree of the final fold can start
                                # as soon as chunks 0..7 are exp'd
                                half1 = emit_half_fold(probsT, 0)
                    else:
                        kc = 0
                        while kc < ntb:
                            if kc % 3 == 0 and kc + 1 < ntb:
                                emit_exp_pair(probsT, k_t, qs_ap, kc)
                                kc += 2
                            else:
                                s_ps = psp2.tile(
                                    [P, 512], F32, tag="spsS", bufs=1,
                                    name="s_ps_s",
                                )
                                nc.tensor.matmul(
                                    s_ps[:], k_t[:, kc, :], qs_ap,
                                    start=True, stop=True,
                                )
                                nc.scalar.activation(
                                    out=probsT[:, kc, :],
                                    in_=s_ps[:],
                                    func=mybir.ActivationFunctionType.Exp,
                                    scale=SCALE,
                                )
                                kc += 1
                    def emit_folds():
                        cur = probsT[:].rearrange("p a b -> p (a b)")
                        width = ntb * QG
                        lvl = 0
                        while width > QG:
                            width //= 2
                            # final level double-buffered: its output is read
                            # by the deferred scol of this qgroup while the
                            # next qgroup's folds already run
                            nxt = p2.tile(
                                [P, width], BF16, tag=f"fold{lvl}",
                                bufs=(2 if width == QG else 1), name="fold",
                            )
                            nc.vector.tensor_tensor(
                                nxt[:],
                                cur[:, 0:width],
                                cur[:, width : 2 * width],
                                add,
                            )
                            cur = nxt[:]
                            lvl += 1
                        return cur

                    # folds (DVE) emitted first so they start right after the
                    # last exp; AV (PE) runs concurrently. For the very last
                    # qgroup the AV eviction goes first so the output DMA
                    # overlaps the fold chain.
                    if not last:
                        sums = emit_folds()
                    if not big:
                        av_ps = psp2.tile(
                            [P, QG], F32, tag="av", bufs=1, name="av_ps"
                        )
                        for kc in range(ntb):
                            nc.tensor.matmul(
                                av_ps[:],
                                v_t[:, kc, :],
                                probsT[:, kc, :],
                                start=(kc == 0),
                                stop=(kc == ntb - 1),
                            )
                    # evac AV (psum->sbuf), ship unnormalized to host.
                    # b0 qgroups run while ACT is saturated with their exps,
                    # but DVE is also folding — split: b0 on ACT, b1 (tail,
                    # ACT-bound) on DVE.
                    av_sb = p2.tile([P, QG], F32, tag="avsb", name="av_sb")
                    if b == 0:
                        nc.scalar.activation(
                            out=av_sb[:],
                            in_=av_ps[:],
                            func=mybir.ActivationFunctionType.Copy,
                        )
                    else:
                        nc.vector.tensor_copy(av_sb[:], av_ps[:])
                    nc.sync.dma_start(
                        out=av_ext[b, hl, :, qg * QG : (qg + 1) * QG],
                        in_=av_sb[:],
                    )
                    if last:
                        # second half-tree + final add (first half ran during
                        # the exp stream)
                        half2 = emit_half_fold(probsT, 1)
                        ff = p2.tile([P, QG], BF16, tag="foldF", bufs=1, name="ff")
                        nc.vector.tensor_tensor(ff[:], half1, half2, add)
                        sums = ff[:]

                    # partition-sums go through the (phase-1) tp bank via a
                    # f32 view — or the idle spsS slot in the tail; deferred
                    # one qgroup so the fold chain never stalls the PE
                    def scol_tail():
                        if tail_scol:
                            # psp1 (tp bank) is closed in the tail; a
                            # dedicated recycled bank stages the sums
                            st = pools["psp3"].tile(
                                [P, 4], F32, tag="scolT", bufs=1, name="scol_t"
                            )
                            scol = st[:]
                        else:
                            tp = pools["psp1"].tile(
                                [P, P], BF16, tag="tp", bufs=1, name="tp_s"
                            )
                            scol = tp[:, 0:8].bitcast(F32)
                        for qs in range(gq):
                            nc.tensor.matmul(
                                scol[:, qs : qs + 1],
                                sums[:, qs * P : (qs + 1) * P],
                                ones_col[:],
                                start=True,
                                stop=True,
                                skip_group_check=True,
                            )
                        nc.vector.tensor_copy(
                            sums_sb[(b, hl)][:, qg * gq : (qg + 1) * gq],
                            scol[:, 0:gq],
                        )
                        if qg == qg_per - 1:
                            nc.sync.dma_start(
                                out=sums_ext[b, hl, :, :], in_=sums_sb[(b, hl)][:]
                            )

                    return scol_tail

                # interleaved emission: batch b's projection tiles are woven
                # between batch b-1's attention qgroups so PE-heavy and
                # ACT-heavy work stay concurrently available to the scheduler
                p2_units = {
                    b: [(b, hl, qg) for qg in range(qg_per) for hl in range(HPC)]
                    for b in range(batches)
                }
                pending_b = None
                pending_s = None

                def emit_tile(b_idx, ti):
                    nonlocal pending_b
                    nxt = phase1_tile(b_idx, ti)
                    if pending_b is not None:
                        pending_b()
                    pending_b = nxt

                def emit_qgroup(u, last=False, big=False, tail_scol=False):
                    nonlocal pending_s
                    nxt = phase2_qgroup(*u, last=last, big=big, tail_scol=tail_scol)
                    if pending_s is not None:
                        pending_s()
                    pending_s = nxt

                with tc.tile_pool(name="psp1", bufs=1, space="PSUM") as psp1:
                    pools["psp1"] = psp1
                    for ti in range(ntb):
                        emit_tile(0, ti)
                    for b in range(1, batches):
                        prev = p2_units[b - 1]
                        ratio = max(1, ntb // max(1, len(prev)))
                        pi = 0
                        for ti in range(ntb):
                            emit_tile(b, ti)
                            if (ti + 1) % ratio == 0 and pi < len(prev):
                                emit_qgroup(prev[pi])
                                pi += 1
                        while pi < len(prev):
                            emit_qgroup(prev[pi])
                            pi += 1
                    if pending_b is not None:
                        pending_b()
                        pending_b = None
                    # last middle scol uses the tp bank — flush before close
                    if pending_s is not None:
                        pending_s()
                        pending_s = None
                with tc.tile_pool(name="psp3", bufs=1, space="PSUM") as psp3:
                    pools["psp3"] = psp3
                    tail_units = p2_units[batches - 1]
                    for ui, u in enumerate(tail_units):
                        emit_qgroup(
                            u, last=(ui == len(tail_units) - 1), tail_scol=True
                        )
                    if pending_s is not None:
                        pending_s()
                        pending_s = None

    nc.compile()
    return nc


def prep_inputs(x, w_qkv, b_qkv, cos, sin):
    """Build per-core input maps (host-side sharding)."""
    bf16 = ml_dtypes.bfloat16
    batches, seq, dim = x.shape
    xt = np.ascontiguousarray(
        x.reshape(batches * seq, dim).T.astype(bf16)
    )  # [DIM, tokens]
    cosf = cos.astype(np.float32)
    sinf = sin.astype(np.float32)
    csf = np.ascontiguousarray(np.concatenate([cosf, sinf], axis=1).astype(bf16))
    scf = np.ascontiguousarray(np.concatenate([-sinf, cosf], axis=1).astype(bf16))
    dperm = np.concatenate([np.arange(0, HEAD_DIM, 2), np.arange(1, HEAD_DIM, 2)])
    dnat = np.arange(HEAD_DIM)
    in_maps = []
    for c in range(NCORES):
        h0, h1 = HPC * c, HPC * c + 1
        idx = np.concatenate(
            [
                h0 * 384 + dperm * 3 + 0,
                h1 * 384 + dperm * 3 + 0,
                h0 * 384 + dperm * 3 + 1,
                h1 * 384 + dperm * 3 + 1,
                h0 * 384 + dnat * 3 + 2,
                h1 * 384 + dnat * 3 + 2,
            ]
        )
        wt = np.ascontiguousarray(w_qkv[idx, :].T.astype(bf16))  # [DIM, 768]
        bb = np.ascontiguousarray(b_qkv[idx].astype(np.float32)[None, :])
        in_maps.append(
            {"xt": xt, "wt": wt, "bias": bb, "cs": csf, "sc": scf}
        )
    return in_maps


_CACHED = {}


def _get_nc(seq, batches):
    key = (seq, batches)
    if key not in _CACHED:
        _CACHED[key] = build_nc(seq, batches)
    return _CACHED[key]


def run(x, w_qkv, b_qkv, cos, sin, trace=False):
    from concourse.bass_utils import run_bass_kernel_spmd

    batches, seq, _ = x.shape
    ntb = seq // P
    nc = _get_nc(seq, batches)
    in_maps = prep_inputs(x, w_qkv, b_qkv, cos, sin)
    res = run_bass_kernel_spmd(
        nc, in_maps, core_ids=list(range(NCORES)), trace=trace
    )
    out = np.empty((batches, seq, NCORES * HPC * HEAD_DIM), dtype=np.float32)
    for c in range(NCORES):
        av = res.results[c]["av"]  # [B, HPC, D, seq]
        sums = res.results[c]["sums"]  # [B, HPC, P, ntb]
        for b in range(batches):
            for hl in range(HPC):
                s = sums[b, hl].T.reshape(seq)  # q = j*128 + p
                h = (HPC * c + hl) * HEAD_DIM
                out[b, :, h : h + HEAD_DIM] = (av[b, hl] / s[None, :]).T
    return out, res


def kernel(x, w_qkv, b_qkv, cos, sin):
    out, _ = run(
        np.asarray(x),
        np.asarray(w_qkv),
        np.asarray(b_qkv),
        np.asarray(cos),
        np.asarray(sin),
        trace=False,
    )
    return out
